# revision 54
# baseline (speedup 1.0000x reference)
# Fused attention block (LeViT-style) for Trainium2, 8 NeuronCores, data-parallel over batch.
#
# reference computation (B=16, N=784, DIM=512, H=8, KD=64, VD=256):
#   qkv = BN(x @ qkv_w.T); split q,k,v per head
#   attn = softmax(q @ k.T * KD**-0.5 + attention_biases[:, bias_idxs])
#   out  = BN(silu(attn @ v reshaped) @ proj_w.T)
#
# Strategy:
#  - batch-parallel: 2 batches per core, weights/bias tables replicated, no collectives
#  - BN folded into weights on host; softmax scale folded into q weights
#  - all matmul operands bf16 (PSUM accumulation fp32), softmax pipeline fp32
#  - scores computed transposed (S^T[j,i]); bias table is symmetric so bias adds unchanged
#  - softmax denominator from an extra ones-column in v (col 256 of each head block)
#  - unstabilized softmax (scores empirically bounded ~|10|, exp is safe in fp32)
#  - every attention matmul runs at uniform PE tile config (128,128): q/k heads are
#    embedded in zero-padded 128-partition blocks (even head rows 0:64, odd rows
#    64:128), and the j/i tails are zero-padded to 896 so no (64,*) or (*,32)
#    configs appear (config switches cost ~100-300ns each)
#  - exp(S)*exp(bias) == exp(S+bias): host precomputes exp(bias) table; the
#    multiply is split across DVE and the otherwise-idle Pool/GpSimd engine
#  - pass A of batch 1 is emitted between the head-pair phases of batch 0's
#    attention, and batch 0's projection chunks alternate with batch 1's pass B,
#    so the PE has filler work during exp-gated pipeline gaps
#  - attention-phase softmax work is spread across engines: exp on ACT, bias
#    multiply split 9:5 DVE/Pool (non-in-place; in-place DVE RMW is ~2x slower),
#    normalize + psum evictions on DVE, proj transpose evicts alternate ACT/DVE
#  - silu applied per t-chunk at the head of each proj chunk (table loads stay 4)

import numpy as np
import ml_dtypes

B, N, DIM = 16, 784, 512
H, KD, VD = 8, 64, 256
RES = 28
EPS = 1e-5
SCALE = KD ** -0.5
NCORES = 8
BL = B // NCORES          # batches per core
VDA = VD + 1              # v head block with ones column
OVW = H * VDA             # 2056
NJP = 896                 # padded j extent (7 * 128)

# t/j chunking over N=784: six 128-chunks + one 16-chunk
CHUNKS = [(i * 128, min(128, N - i * 128)) for i in range((N + 127) // 128)]
ITILES = [(0, 512), (512, N - 512)]   # free-dim tiles for 784 (<=512 per PSUM bank)

_CACHE = {}


def _build_nc():
    from contextlib import ExitStack
    import concourse.bacc as bacc
    import concourse.tile as tile
    from concourse import mybir

    bf = mybir.dt.bfloat16
    f8 = mybir.dt.float8e4
    f32 = mybir.dt.float32
    AF = mybir.ActivationFunctionType
    MULT = mybir.AluOpType.mult
    ADD = mybir.AluOpType.add

    nc = bacc.Bacc("TRN2", target_bir_lowering=False, debug=False)

    xT = nc.dram_tensor("xT", [BL, DIM, N], bf, kind="ExternalInput").ap()
    wqk = nc.dram_tensor("wqk", [128, 4, 1024], bf, kind="ExternalInput").ap()
    wv = nc.dram_tensor("wv", [128, 4, H * VD], bf, kind="ExternalInput").ap()
    wp = nc.dram_tensor("wp", [128, 16, DIM], bf, kind="ExternalInput").ap()
    bqk = nc.dram_tensor("bqk", [128, 8], f32, kind="ExternalInput").ap()
    bv = nc.dram_tensor("bv", [128, H * VD], bf, kind="ExternalInput").ap()
    bp = nc.dram_tensor("bp", [128, DIM], bf, kind="ExternalInput").ap()
    biast = nc.dram_tensor("biast", [H, NJP, N], bf, kind="ExternalInput").ap()
    ident = nc.dram_tensor("ident", [128, 128], bf, kind="ExternalInput").ap()
    out = nc.dram_tensor("out", [BL, N, DIM], f32, kind="ExternalOutput").ap()

    import concourse.bass as bass

    with ExitStack() as ctx:
        tc = ctx.enter_context(tile.TileContext(nc))
        consts = ctx.enter_context(tc.tile_pool(name="consts", bufs=1))
        xpool = ctx.enter_context(tc.tile_pool(name="xpool", bufs=1))
        qkpool = ctx.enter_context(tc.tile_pool(name="qkpool", bufs=1))
        tevpool = ctx.enter_context(tc.tile_pool(name="tevpool", bufs=3))
        vpool = ctx.enter_context(tc.tile_pool(name="vpool", bufs=1))
        silupool = ctx.enter_context(tc.tile_pool(name="silupool", bufs=1))
        biaspool = ctx.enter_context(tc.tile_pool(name="biaspool", bufs=6))
        eppool = ctx.enter_context(tc.tile_pool(name="eppool", bufs=4))
        ppool = ctx.enter_context(tc.tile_pool(name="ppool", bufs=3))
        smalls = ctx.enter_context(tc.tile_pool(name="smalls", bufs=4))
        tpool = ctx.enter_context(tc.tile_pool(name="tpool", bufs=4))
        fopool = ctx.enter_context(tc.tile_pool(name="fopool", bufs=2))
        # PSUM budget (8 banks): psbig 3x2 + pssm 2x1 = 8
        psbig = ctx.enter_context(tc.tile_pool(name="psbig", bufs=3, space="PSUM"))
        pssm = ctx.enter_context(tc.tile_pool(name="pssm", bufs=2, space="PSUM"))

        # ---- x(0) and wqk first, split per contraction chunk so the first
        # pass A matmuls start as soon as their slices land ----
        x_sb = [None, None]
        x_sb[0] = xpool.tile([128, 4, N], bf, name="x_0", tag="x")
        wqk_sb = consts.tile([128, 4, 1024], bf)
        for cc in range(4):
            nc.sync.dma_start(
                out=x_sb[0][:, cc, :],
                in_=bass.AP(tensor=xT.tensor, offset=xT.offset + cc * 128 * N,
                            ap=[[N, 128], [1, N]]),
            )
            nc.sync.dma_start(out=wqk_sb[:, cc, :], in_=wqk[:, cc, :])
        wv_sb = consts.tile([128, 4, H * VD], bf)
        nc.sync.dma_start(out=wv_sb, in_=wv)
        wp_sb = consts.tile([128, 16, DIM], bf)
        nc.sync.dma_start(out=wp_sb, in_=wp)
        bqk_sb = consts.tile([128, 8], f32)
        nc.sync.dma_start(out=bqk_sb, in_=bqk)
        bp_sb = consts.tile([128, DIM], bf)
        nc.sync.dma_start(out=bp_sb, in_=bp)
        ident_sb = consts.tile([128, 128], bf)
        nc.sync.dma_start(out=ident_sb, in_=ident)
        bv_sb = consts.tile([128, H * VD], bf)
        nc.sync.dma_start(out=bv_sb, in_=bv)

        # qk_pad[*, 2h, :] = q_h, qk_pad[*, 2h+1, :] = k_h; head h occupies
        # partitions (h%2)*64:(h%2+1)*64, everything else stays zero so all
        # score matmuls are uniform K=128, with j-windows up to 896 (cols
        # 784:896 zero) so the j-tail is M=128 too.
        qk_pad = qkpool.tile([128, 16, NJP], bf)
        nc.gpsimd.memset(qk_pad, 0.0)

        # v_sb allocated once: tail rows 16:128 of the last t-chunk are never
        # written by pass B; zero them once so K=128 AV j-tail matmuls stream
        # 0*0 instead of 0*garbage (avoids NaN propagation).
        v_sb = vpool.tile([128, 7, OVW], bf)
        v_resh = v_sb.rearrange("p t (h d) -> p t h d", d=VDA)
        nc.gpsimd.memset(v_sb[:, 6, :], 0.0)
        nc.gpsimd.memset(v_resh[:, :, :, VD:VDA], 1.0)

        silu_sb = [None, None]

        def load_x(b):
            x_t = xpool.tile([128, 4, N], bf, name=f"x_{b}", tag="x")
            xin = bass.AP(
                tensor=xT.tensor,
                offset=xT.offset + b * DIM * N,
                ap=[[N, 128], [128 * N, 4], [1, N]],
            )
            nc.sync.dma_start(out=x_t, in_=xin)
            x_sb[b] = x_t

        def pass_a_oc(b, oc, pool):
            # one o-chunk of qkT: evict to transient, DMA the two head-halves
            # into their zero-padded qk_pad blocks (same partition range)
            tev = tevpool.tile([128, N], bf, name=f"tev_{b}_{oc}", tag="tev")
            ps = pool.tile([128, N], f32, name=f"pa_{b}_{oc}", tag="big")
            for (i0, isz) in ITILES:
                for cc in range(4):
                    nc.tensor.matmul(
                        ps[:, i0:i0 + isz],
                        lhsT=wqk_sb[:, cc, oc * 128:(oc + 1) * 128],
                        rhs=x_sb[b][:, cc, i0:i0 + isz],
                        start=(cc == 0),
                        stop=(cc == 3),
                    )
            nc.vector.tensor_scalar_add(
                out=tev, in0=ps, scalar1=bqk_sb[:, oc:oc + 1],
            )
            isq = 0 if oc < 4 else 1  # q chunks first, then k
            for half in range(2):
                h = (oc % 4) * 2 + half
                blk = 2 * h + isq
                nc.sync.dma_start(
                    out=qk_pad[half * 64:(half + 1) * 64, blk, :N],
                    in_=tev[half * 64:(half + 1) * 64, :],
                )

        def pass_b_ovt(b, ovt):
                # all 7 t-chunks for one head-pair column group; writes only
                # the v regions that AV(b-1, hp=ovt) has finished reading
                o0 = ovt * 512
                for tc_i, (t0, tsz) in enumerate(CHUNKS):
                    ps = pssm.tile([128, 512], f32, tag="small")
                    for cc in range(4):
                        nc.tensor.matmul(
                            ps[:tsz, :],
                            lhsT=x_sb[b][:, cc, t0:t0 + tsz],
                            rhs=wv_sb[:, cc, o0:o0 + 512],
                            start=(cc == 0),
                            stop=(cc == 3),
                        )
                    nc.vector.tensor_tensor(
                        out=v_resh[:tsz, tc_i, 2 * ovt:2 * ovt + 2, :VD],
                        in0=ps[:tsz, :].rearrange("p (h d) -> p h d", d=VD),
                        in1=bv_sb[:tsz, o0:o0 + 512].rearrange(
                            "p (h d) -> p h d", d=VD
                        ),
                        op=ADD,
                    )

        def pass_b_tc(b, tc_i):
                t0, tsz = CHUNKS[tc_i]
                for ovt in range(4):
                    o0 = ovt * 512
                    ps = pssm.tile([128, 512], f32, tag="small")
                    for cc in range(4):
                        nc.tensor.matmul(
                            ps[:tsz, :],
                            lhsT=x_sb[b][:, cc, t0:t0 + tsz],
                            rhs=wv_sb[:, cc, o0:o0 + 512],
                            start=(cc == 0),
                            stop=(cc == 3),
                        )
                    # strided evict into the two 257-stride head blocks; bias
                    # added here (DVE) instead of a K=1 seed matmul on the PE
                    nc.vector.tensor_tensor(
                        out=v_resh[:tsz, tc_i, 2 * ovt:2 * ovt + 2, :VD],
                        in0=ps[:tsz, :].rearrange("p (h d) -> p h d", d=VD),
                        in1=bv_sb[:tsz, o0:o0 + 512].rearrange(
                            "p (h d) -> p h d", d=VD
                        ),
                        op=ADD,
                    )

        def attn_hp(b, hp, p_tail_memset):
            p_par = []
            for k in range(2):
                p_sb = ppool.tile([128, 7, NJP], bf, name=f"p_{b}_{hp}_{k}", tag="p")
                p_par.append(p_sb)
                # zero the i-tail cols once per physical buffer (3-buf round
                # robin; first three allocations cover all bufs) so AV's last
                # i-chunk can run M=128 over the padded window
                if p_tail_memset and (hp, k) in ((0, 0), (0, 1), (1, 0)):
                    nc.gpsimd.memset(p_sb[:, :, N:NJP], 0.0)

            for jc, (j0, jsz) in enumerate(CHUNKS):
                pse = psbig.tile([128, N], f32, tag="big")
                pso = psbig.tile([128, N], f32, tag="big")
                for (i0, isz) in ITILES:
                    for k, ps in ((0, pse), (1, pso)):
                        h = 2 * hp + k
                        nc.tensor.matmul(
                            ps[:, i0:i0 + isz],
                            lhsT=qk_pad[:, 2 * h + 1, j0:j0 + 128],
                            rhs=qk_pad[:, 2 * h, i0:i0 + isz],
                            start=True, stop=True,
                        )
                for k, ps in ((0, pse), (1, pso)):
                    h = 2 * hp + k
                    bias_sb = biaspool.tile([128, N], bf, tag="bt")
                    bin_ = bass.AP(
                        tensor=biast.tensor,
                        offset=biast.offset + (h * NJP + jc * 128) * N,
                        ap=[[N, 128], [1, N]],
                    )
                    nc.sync.dma_start(out=bias_sb, in_=bin_)
                    ep = eppool.tile([128, N], bf, tag="ep")
                    # exp(S)*exp(bias) == exp(S+bias); biast holds exp(bias),
                    # rows >= 784 zero so padded j-tail rows of P come out zero
                    nc.scalar.activation(out=ep, in_=ps[:, :], func=AF.Exp)
                    # non-in-place multiply (in-place RMW runs ~2x slower on
                    # DVE); split across DVE and Pool 9:5 per head pair
                    eng = nc.vector if (k == 0 or jc >= 5) else nc.gpsimd
                    eng.tensor_tensor(
                        out=p_par[k][:, jc, :N], in0=ep, in1=bias_sb, op=MULT,
                    )

            for k in range(2):
                h = 2 * hp + k
                for ic, (i0, isz) in enumerate(CHUNKS):
                    ps = pssm.tile([128, 512], f32, tag="small")
                    for jc, (j0, jsz) in enumerate(CHUNKS):
                        # M=128 always (i-tail reads the zero-padded window;
                        # garbage rows of psum are never read)
                        nc.tensor.matmul(
                            ps[:, :VDA],
                            lhsT=p_par[k][:, jc, i0:i0 + 128],
                            rhs=v_sb[:, jc, h * VDA:(h + 1) * VDA],
                            start=(jc == 0),
                            stop=(jc == 6),
                        )
                    rs = smalls.tile([128, 1], f32)
                    nc.vector.reciprocal(out=rs[:isz], in_=ps[:isz, VD:VDA])
                    # normalized pre-silu values (silu applied in bulk later)
                    nc.vector.tensor_scalar_mul(
                        out=silu_sb[b][:isz, ic, h * VD:(h + 1) * VD],
                        in0=ps[:isz, :VD], scalar1=rs[:isz, 0:1],
                    )

        def proj_tc(b, tc_i, unused=None):
            t0, tsz = CHUNKS[tc_i]
            nc.scalar.activation(
                out=silu_sb[b][:tsz, tc_i, :], in_=silu_sb[b][:tsz, tc_i, :],
                func=AF.Silu,
            )
            psf = pssm.tile([128, 512], f32, tag="small")
            for vp in range(8):
                pst = psbig.tile(
                    [128, 2, 128], bf, name=f"pst_{b}_{tc_i}_{vp}", tag="big",
                )
                for k in range(2):
                    vc = vp * 2 + k
                    nc.tensor.transpose(
                        pst[:, k, :tsz],
                        silu_sb[b][:tsz, tc_i, vc * 128:(vc + 1) * 128],
                        ident_sb[:tsz, :tsz],
                    )
                st = tpool.tile([128, 2, 128], bf)
                # psum -> SBUF evict, alternating ACT/DVE
                if vp % 2 == 0:
                    nc.scalar.copy(out=st[:, :, :tsz], in_=pst[:, :, :tsz])
                else:
                    nc.vector.tensor_copy(out=st[:, :, :tsz], in_=pst[:, :, :tsz])
                for k in range(2):
                    vc = vp * 2 + k
                    nc.tensor.matmul(
                        psf[:tsz, :],
                        lhsT=st[:, k, :tsz],
                        rhs=wp_sb[:, vc, :],
                        start=(vc == 0),
                        stop=(vc == 15),
                    )
            fo = fopool.tile([128, DIM], f32)
            nc.vector.tensor_tensor(
                out=fo[:tsz], in0=psf[:tsz], in1=bp_sb[:tsz], op=ADD,
            )
            nc.sync.dma_start(out=out[b, t0:t0 + tsz, :], in_=fo[:tsz])

        # ---- schedule: passA(b1) interleaved into attn(b0) head-pair
        # phases (fills exp-gated PE gaps); rest sequential ----
        for oc in range(8):
            pass_a_oc(0, oc, psbig)
        for tc_i in range(7):
            pass_b_tc(0, tc_i)
        silu_sb[0] = silupool.tile([128, 7, H * VD], bf, name="silu_0", tag="silu")
        load_x(1)
        for hp in range(4):
            attn_hp(0, hp, p_tail_memset=True)
            pass_a_oc(1, hp, psbig)
            pass_a_oc(1, 4 + hp, psbig)
        for tc_i in range(7):
            proj_tc(0, tc_i)
            pass_b_tc(1, tc_i)
        silu_sb[1] = silupool.tile([128, 7, H * VD], bf, name="silu_1", tag="silu")
        for hp in range(4):
            attn_hp(1, hp, p_tail_memset=False)
        for tc_i in range(7):
            proj_tc(1, tc_i)

    nc.finalize()
    return nc


def _prep(inputs):
    bf16 = ml_dtypes.bfloat16
    f8 = ml_dtypes.float8_e4m3fn
    f32 = np.float32
    inputs = {k: np.asarray(v) for k, v in inputs.items()}

    s_qkv = (inputs["qkv_gamma"] / np.sqrt(inputs["qkv_var"] + EPS)).astype(f32)
    b_qkv = (inputs["qkv_beta"] - inputs["qkv_mean"] * s_qkv).astype(f32)
    w_fold = (inputs["qkv_w"] * s_qkv[:, None]).astype(f32)

    rows = np.arange((2 * KD + VD) * H).reshape(H, 2 * KD + VD)
    q_rows = rows[:, :KD].ravel()
    k_rows = rows[:, KD:2 * KD].ravel()
    v_rows = rows[:, 2 * KD:].ravel()

    wq = w_fold[q_rows] * SCALE
    bq = b_qkv[q_rows] * SCALE
    wk = w_fold[k_rows]
    bk = b_qkv[k_rows]
    wvm = w_fold[v_rows]
    bvm = b_qkv[v_rows]

    # wqk: [c, o] with o = [q(512), k(512)] -> [128, cc, 1024]
    wqkT = np.concatenate([wq, wk], axis=0).T.astype(bf16)          # [512, 1024]
    wqk_t = np.ascontiguousarray(wqkT.reshape(4, 128, 1024).transpose(1, 0, 2))
    bqk_t = np.concatenate([bq, bk]).reshape(8, 128).T.astype(f32)  # [128, 8]
    bqk_t = np.ascontiguousarray(bqk_t)

    wv_t = np.ascontiguousarray(
        wvm.T.astype(bf16).reshape(4, 128, H * VD).transpose(1, 0, 2)
    )
    bv_t = np.ascontiguousarray(
        np.broadcast_to(bvm.astype(bf16)[None, :], (128, H * VD))
    )

    s_p = (inputs["proj_gamma"] / np.sqrt(inputs["proj_var"] + EPS)).astype(f32)
    b_p = (inputs["proj_beta"] - inputs["proj_mean"] * s_p).astype(f32)
    wp_fold = (inputs["proj_w"] * s_p[:, None]).astype(f32)          # [512, 2048]
    wp_t = np.ascontiguousarray(
        wp_fold.T.astype(bf16).reshape(16, 128, DIM).transpose(1, 0, 2)
    )
    bp_t = np.ascontiguousarray(
        np.broadcast_to(b_p.astype(bf16)[None, :], (128, DIM))
    )

    bias_full = inputs["attention_biases"][:, inputs["bias_idxs"]].astype(f32)  # [H, N, N]
    biast = np.zeros((H, NJP, N), dtype=bf16)
    biast[:, :N, :] = np.exp(bias_full).astype(bf16)   # multiplicative form

    xT = inputs["x"].transpose(0, 2, 1).astype(bf16)                 # [B, 512, 784]

    shared = {
        "wqk": wqk_t, "wv": wv_t, "wp": wp_t, "bqk": bqk_t,
        "bv": bv_t, "bp": bp_t, "biast": biast,
        "ident": np.eye(128, dtype=np.float32).astype(bf16),
    }
    in_maps = []
    for c in range(NCORES):
        m = dict(shared)
        m["xT"] = np.ascontiguousarray(xT[c * BL:(c + 1) * BL])
        in_maps.append(m)
    return in_maps


def kernel(trace=False, **inputs):
    from concourse import bass_utils

    if "nc" not in _CACHE:
        _CACHE["nc"] = _build_nc()
    nc = _CACHE["nc"]

    in_maps = _prep(inputs)
    res = bass_utils.run_bass_kernel_spmd(
        nc, in_maps, core_ids=list(range(NCORES)), trace=trace,
    )
    out = np.concatenate([r["out"] for r in res.results], axis=0)
    if trace:
        return out.astype(np.float32), res
    return out.astype(np.float32)


# revision 55
# speedup vs baseline: 1.1918x; 1.1918x over previous
# Fused attention block (LeViT-style) for Trainium2, 8 NeuronCores, data-parallel over batch.
#
# reference computation (B=16, N=784, DIM=512, H=8, KD=64, VD=256):
#   qkv = BN(x @ qkv_w.T); split q,k,v per head
#   attn = softmax(q @ k.T * KD**-0.5 + attention_biases[:, bias_idxs])
#   out  = BN(silu(attn @ v reshaped) @ proj_w.T)
#
# Strategy:
#  - batch-parallel: 2 batches per core, weights/bias tables replicated, no collectives
#  - BN folded into weights on host; softmax scale folded into q weights
#  - all matmul operands bf16 (PSUM accumulation fp32), softmax pipeline fp32
#  - scores computed transposed (S^T[j,i]); bias table is symmetric so bias adds unchanged
#  - softmax denominator from an extra ones-column in v (col 256 of each head block)
#  - unstabilized softmax (scores empirically bounded ~|10|, exp is safe in fp32)
#  - every attention matmul runs at uniform PE tile config (128,128): q/k heads are
#    embedded in zero-padded 128-partition blocks (even head rows 0:64, odd rows
#    64:128), and the j/i tails are zero-padded to 896 so no (64,*) or (*,32)
#    configs appear (config switches cost ~100-300ns each)
#  - exp(S)*exp(bias) == exp(S+bias): host precomputes exp(bias) table; the
#    multiply is split across DVE and the otherwise-idle Pool/GpSimd engine
#  - pass A of batch 1 is emitted between the head-pair phases of batch 0's
#    attention, and batch 0's projection chunks alternate with batch 1's pass B,
#    so the PE has filler work during exp-gated pipeline gaps
#  - attention-phase softmax work is spread across engines: exp on ACT, bias
#    multiply split 9:5 DVE/Pool (non-in-place; in-place DVE RMW is ~2x slower),
#    normalize + psum evictions on DVE, proj transpose evicts alternate ACT/DVE
#  - silu applied per t-chunk at the head of each proj chunk (table loads stay 4)

import numpy as np
import ml_dtypes

B, N, DIM = 16, 784, 512
H, KD, VD = 8, 64, 256
RES = 28
EPS = 1e-5
SCALE = KD ** -0.5
NCORES = 8
BL = B // NCORES          # batches per core
VDA = VD + 1              # v head block with ones column
OVW = H * VDA             # 2056
NJP = 896                 # padded j extent (7 * 128)

# t/j chunking over N=784: six 128-chunks + one 16-chunk
CHUNKS = [(i * 128, min(128, N - i * 128)) for i in range((N + 127) // 128)]
ITILES = [(0, 512), (512, N - 512)]   # free-dim tiles for 784 (<=512 per PSUM bank)

_CACHE = {}


def _build_nc():
    from contextlib import ExitStack
    import concourse.bacc as bacc
    import concourse.tile as tile
    from concourse import mybir

    bf = mybir.dt.bfloat16
    f8 = mybir.dt.float8e4
    f32 = mybir.dt.float32
    AF = mybir.ActivationFunctionType
    MULT = mybir.AluOpType.mult
    ADD = mybir.AluOpType.add

    nc = bacc.Bacc("TRN2", target_bir_lowering=False, debug=False)

    xT = nc.dram_tensor("xT", [BL, DIM, N], bf, kind="ExternalInput").ap()
    wqk = nc.dram_tensor("wqk", [128, 4, 1024], bf, kind="ExternalInput").ap()
    wv = nc.dram_tensor("wv", [128, 4, H * VD], bf, kind="ExternalInput").ap()
    wp = nc.dram_tensor("wp", [128, 16, DIM], bf, kind="ExternalInput").ap()
    bqk = nc.dram_tensor("bqk", [128, 8], f32, kind="ExternalInput").ap()
    bv = nc.dram_tensor("bv", [128, H * VD], bf, kind="ExternalInput").ap()
    bp = nc.dram_tensor("bp", [128, DIM], bf, kind="ExternalInput").ap()
    biast = nc.dram_tensor("biast", [H, NJP, N], bf, kind="ExternalInput").ap()
    ident = nc.dram_tensor("ident", [128, 128], bf, kind="ExternalInput").ap()
    out = nc.dram_tensor("out", [BL, N, DIM], f32, kind="ExternalOutput").ap()

    import concourse.bass as bass

    with ExitStack() as ctx:
        tc = ctx.enter_context(tile.TileContext(nc))
        consts = ctx.enter_context(tc.tile_pool(name="consts", bufs=1))
        xpool = ctx.enter_context(tc.tile_pool(name="xpool", bufs=1))
        qkpool = ctx.enter_context(tc.tile_pool(name="qkpool", bufs=1))
        tevpool = ctx.enter_context(tc.tile_pool(name="tevpool", bufs=3))
        vpool = ctx.enter_context(tc.tile_pool(name="vpool", bufs=1))
        silupool = ctx.enter_context(tc.tile_pool(name="silupool", bufs=1))
        biaspool = ctx.enter_context(tc.tile_pool(name="biaspool", bufs=6))
        eppool = ctx.enter_context(tc.tile_pool(name="eppool", bufs=4))
        ppool = ctx.enter_context(tc.tile_pool(name="ppool", bufs=3))
        smalls = ctx.enter_context(tc.tile_pool(name="smalls", bufs=4))
        tpool = ctx.enter_context(tc.tile_pool(name="tpool", bufs=4))
        fopool = ctx.enter_context(tc.tile_pool(name="fopool", bufs=2))
        # PSUM budget (8 banks): psbig 3x2 + pssm 2x1 = 8
        psbig = ctx.enter_context(tc.tile_pool(name="psbig", bufs=3, space="PSUM"))
        pssm = ctx.enter_context(tc.tile_pool(name="pssm", bufs=2, space="PSUM"))

        # ---- x(0) and wqk first: pass A needs only these two ----
        x_sb = [None, None]
        x_sb[0] = xpool.tile([128, 4, N], bf, name="x_0", tag="x")
        nc.sync.dma_start(
            out=x_sb[0],
            in_=bass.AP(tensor=xT.tensor, offset=xT.offset,
                        ap=[[N, 128], [128 * N, 4], [1, N]]),
        )
        wqk_sb = consts.tile([128, 4, 1024], bf)
        nc.sync.dma_start(out=wqk_sb, in_=wqk)
        wv_sb = consts.tile([128, 4, H * VD], bf)
        nc.sync.dma_start(out=wv_sb, in_=wv)
        wp_sb = consts.tile([128, 16, DIM], bf)
        nc.sync.dma_start(out=wp_sb, in_=wp)
        bqk_sb = consts.tile([128, 8], f32)
        nc.sync.dma_start(out=bqk_sb, in_=bqk)
        bp_sb = consts.tile([128, DIM], bf)
        nc.sync.dma_start(out=bp_sb, in_=bp)
        ident_sb = consts.tile([128, 128], bf)
        nc.sync.dma_start(out=ident_sb, in_=ident)
        bv_sb = consts.tile([128, H * VD], bf)
        nc.sync.dma_start(out=bv_sb, in_=bv)

        # qk_pad[*, 2h, :] = q_h, qk_pad[*, 2h+1, :] = k_h; head h occupies
        # partitions (h%2)*64:(h%2+1)*64, everything else stays zero so all
        # score matmuls are uniform K=128, with j-windows up to 896 (cols
        # 784:896 zero) so the j-tail is M=128 too.
        qk_pad = qkpool.tile([128, 16, NJP], bf)
        nc.gpsimd.memset(qk_pad, 0.0)

        # v_sb allocated once: tail rows 16:128 of the last t-chunk are never
        # written by pass B; zero them once so K=128 AV j-tail matmuls stream
        # 0*0 instead of 0*garbage (avoids NaN propagation).
        v_sb = vpool.tile([128, 7, OVW], bf)
        v_resh = v_sb.rearrange("p t (h d) -> p t h d", d=VDA)
        nc.gpsimd.memset(v_sb[:, 6, :], 0.0)
        nc.gpsimd.memset(v_resh[:, :, :, VD:VDA], 1.0)

        silu_sb = [None, None]

        def load_x(b):
            x_t = xpool.tile([128, 4, N], bf, name=f"x_{b}", tag="x")
            xin = bass.AP(
                tensor=xT.tensor,
                offset=xT.offset + b * DIM * N,
                ap=[[N, 128], [128 * N, 4], [1, N]],
            )
            nc.sync.dma_start(out=x_t, in_=xin)
            x_sb[b] = x_t

        def pass_a_oc(b, oc, pool):
            # one o-chunk of qkT: evict to transient, DMA the two head-halves
            # into their zero-padded qk_pad blocks (same partition range)
            tev = tevpool.tile([128, N], bf, name=f"tev_{b}_{oc}", tag="tev")
            ps = pool.tile([128, N], f32, name=f"pa_{b}_{oc}", tag="big")
            for (i0, isz) in ITILES:
                for cc in range(4):
                    nc.tensor.matmul(
                        ps[:, i0:i0 + isz],
                        lhsT=wqk_sb[:, cc, oc * 128:(oc + 1) * 128],
                        rhs=x_sb[b][:, cc, i0:i0 + isz],
                        start=(cc == 0),
                        stop=(cc == 3),
                    )
            nc.vector.tensor_scalar_add(
                out=tev, in0=ps, scalar1=bqk_sb[:, oc:oc + 1],
            )
            isq = 0 if oc < 4 else 1  # q chunks first, then k
            for half in range(2):
                h = (oc % 4) * 2 + half
                blk = 2 * h + isq
                nc.sync.dma_start(
                    out=qk_pad[half * 64:(half + 1) * 64, blk, :N],
                    in_=tev[half * 64:(half + 1) * 64, :],
                )

        def pass_b_ovt(b, ovt):
                # all 7 t-chunks for one head-pair column group; writes only
                # the v regions that AV(b-1, hp=ovt) has finished reading
                o0 = ovt * 512
                for tc_i, (t0, tsz) in enumerate(CHUNKS):
                    ps = pssm.tile([128, 512], f32, tag="small")
                    for cc in range(4):
                        nc.tensor.matmul(
                            ps[:tsz, :],
                            lhsT=x_sb[b][:, cc, t0:t0 + tsz],
                            rhs=wv_sb[:, cc, o0:o0 + 512],
                            start=(cc == 0),
                            stop=(cc == 3),
                        )
                    nc.vector.tensor_tensor(
                        out=v_resh[:tsz, tc_i, 2 * ovt:2 * ovt + 2, :VD],
                        in0=ps[:tsz, :].rearrange("p (h d) -> p h d", d=VD),
                        in1=bv_sb[:tsz, o0:o0 + 512].rearrange(
                            "p (h d) -> p h d", d=VD
                        ),
                        op=ADD,
                    )

        def pass_b_tc(b, tc_i):
                t0, tsz = CHUNKS[tc_i]
                for ovt in range(4):
                    o0 = ovt * 512
                    ps = pssm.tile([128, 512], f32, tag="small")
                    for cc in range(4):
                        nc.tensor.matmul(
                            ps[:tsz, :],
                            lhsT=x_sb[b][:, cc, t0:t0 + tsz],
                            rhs=wv_sb[:, cc, o0:o0 + 512],
                            start=(cc == 0),
                            stop=(cc == 3),
                        )
                    # strided evict into the two 257-stride head blocks; bias
                    # added here (DVE) instead of a K=1 seed matmul on the PE
                    nc.vector.tensor_tensor(
                        out=v_resh[:tsz, tc_i, 2 * ovt:2 * ovt + 2, :VD],
                        in0=ps[:tsz, :].rearrange("p (h d) -> p h d", d=VD),
                        in1=bv_sb[:tsz, o0:o0 + 512].rearrange(
                            "p (h d) -> p h d", d=VD
                        ),
                        op=ADD,
                    )

        def attn_hp(b, hp, p_tail_memset):
            p_par = []
            for k in range(2):
                p_sb = ppool.tile([128, 7, NJP], bf, name=f"p_{b}_{hp}_{k}", tag="p")
                p_par.append(p_sb)
                # zero the i-tail cols once per physical buffer (3-buf round
                # robin; first three allocations cover all bufs) so AV's last
                # i-chunk can run M=128 over the padded window
                if p_tail_memset and (hp, k) in ((0, 0), (0, 1), (1, 0)):
                    nc.gpsimd.memset(p_sb[:, :, N:NJP], 0.0)

            for jc, (j0, jsz) in enumerate(CHUNKS):
                pse = psbig.tile([128, N], f32, tag="big")
                pso = psbig.tile([128, N], f32, tag="big")
                for (i0, isz) in ITILES:
                    for k, ps in ((0, pse), (1, pso)):
                        h = 2 * hp + k
                        nc.tensor.matmul(
                            ps[:, i0:i0 + isz],
                            lhsT=qk_pad[:, 2 * h + 1, j0:j0 + 128],
                            rhs=qk_pad[:, 2 * h, i0:i0 + isz],
                            start=True, stop=True,
                        )
                for k, ps in ((0, pse), (1, pso)):
                    h = 2 * hp + k
                    bias_sb = biaspool.tile([128, N], bf, tag="bt")
                    bin_ = bass.AP(
                        tensor=biast.tensor,
                        offset=biast.offset + (h * NJP + jc * 128) * N,
                        ap=[[N, 128], [1, N]],
                    )
                    nc.sync.dma_start(out=bias_sb, in_=bin_)
                    ep = eppool.tile([128, N], bf, tag="ep")
                    # exp(S)*exp(bias) == exp(S+bias); biast holds exp(bias),
                    # rows >= 784 zero so padded j-tail rows of P come out zero
                    nc.scalar.activation(out=ep, in_=ps[:, :], func=AF.Exp)
                    # non-in-place multiply (in-place RMW runs ~2x slower on
                    # DVE); split across DVE and Pool 9:5 per head pair
                    eng = nc.vector if (k == 0 or jc >= 5) else nc.gpsimd
                    eng.tensor_tensor(
                        out=p_par[k][:, jc, :N], in0=ep, in1=bias_sb, op=MULT,
                    )

            for k in range(2):
                h = 2 * hp + k
                for ic, (i0, isz) in enumerate(CHUNKS):
                    ps = pssm.tile([128, 512], f32, tag="small")
                    for jc, (j0, jsz) in enumerate(CHUNKS):
                        # M=128 always (i-tail reads the zero-padded window;
                        # garbage rows of psum are never read)
                        nc.tensor.matmul(
                            ps[:, :VDA],
                            lhsT=p_par[k][:, jc, i0:i0 + 128],
                            rhs=v_sb[:, jc, h * VDA:(h + 1) * VDA],
                            start=(jc == 0),
                            stop=(jc == 6),
                        )
                    rs = smalls.tile([128, 1], f32)
                    nc.vector.reciprocal(out=rs[:isz], in_=ps[:isz, VD:VDA])
                    # normalized pre-silu values (silu applied in bulk later)
                    nc.vector.tensor_scalar_mul(
                        out=silu_sb[b][:isz, ic, h * VD:(h + 1) * VD],
                        in0=ps[:isz, :VD], scalar1=rs[:isz, 0:1],
                    )

        def proj_tc(b, tc_i, unused=None):
            t0, tsz = CHUNKS[tc_i]
            nc.scalar.activation(
                out=silu_sb[b][:tsz, tc_i, :], in_=silu_sb[b][:tsz, tc_i, :],
                func=AF.Silu,
            )
            psf = pssm.tile([128, 512], f32, tag="small")
            for vp in range(8):
                pst = psbig.tile(
                    [128, 2, 128], bf, name=f"pst_{b}_{tc_i}_{vp}", tag="big",
                )
                for k in range(2):
                    vc = vp * 2 + k
                    nc.tensor.transpose(
                        pst[:, k, :tsz],
                        silu_sb[b][:tsz, tc_i, vc * 128:(vc + 1) * 128],
                        ident_sb[:tsz, :tsz],
                    )
                st = tpool.tile([128, 2, 128], bf)
                # psum -> SBUF evict, alternating ACT/DVE
                if vp % 2 == 0:
                    nc.scalar.copy(out=st[:, :, :tsz], in_=pst[:, :, :tsz])
                else:
                    nc.vector.tensor_copy(out=st[:, :, :tsz], in_=pst[:, :, :tsz])
                for k in range(2):
                    vc = vp * 2 + k
                    nc.tensor.matmul(
                        psf[:tsz, :],
                        lhsT=st[:, k, :tsz],
                        rhs=wp_sb[:, vc, :],
                        start=(vc == 0),
                        stop=(vc == 15),
                    )
            fo = fopool.tile([128, DIM], f32)
            nc.vector.tensor_tensor(
                out=fo[:tsz], in0=psf[:tsz], in1=bp_sb[:tsz], op=ADD,
            )
            nc.sync.dma_start(out=out[b, t0:t0 + tsz, :], in_=fo[:tsz])

        # ---- schedule: passA(b1) interleaved into attn(b0) head-pair
        # phases (fills exp-gated PE gaps); rest sequential ----
        for oc in range(8):
            pass_a_oc(0, oc, psbig)
        for tc_i in range(7):
            pass_b_tc(0, tc_i)
        silu_sb[0] = silupool.tile([128, 7, H * VD], bf, name="silu_0", tag="silu")
        load_x(1)
        for hp in range(4):
            attn_hp(0, hp, p_tail_memset=True)
            pass_a_oc(1, hp, psbig)
            pass_a_oc(1, 4 + hp, psbig)
        for tc_i in range(7):
            proj_tc(0, tc_i)
            pass_b_tc(1, tc_i)
        silu_sb[1] = silupool.tile([128, 7, H * VD], bf, name="silu_1", tag="silu")
        for hp in range(4):
            attn_hp(1, hp, p_tail_memset=False)
        for tc_i in range(7):
            proj_tc(1, tc_i)

    nc.finalize()
    return nc


def _prep(inputs):
    bf16 = ml_dtypes.bfloat16
    f8 = ml_dtypes.float8_e4m3fn
    f32 = np.float32
    inputs = {k: np.asarray(v) for k, v in inputs.items()}

    s_qkv = (inputs["qkv_gamma"] / np.sqrt(inputs["qkv_var"] + EPS)).astype(f32)
    b_qkv = (inputs["qkv_beta"] - inputs["qkv_mean"] * s_qkv).astype(f32)
    w_fold = (inputs["qkv_w"] * s_qkv[:, None]).astype(f32)

    rows = np.arange((2 * KD + VD) * H).reshape(H, 2 * KD + VD)
    q_rows = rows[:, :KD].ravel()
    k_rows = rows[:, KD:2 * KD].ravel()
    v_rows = rows[:, 2 * KD:].ravel()

    wq = w_fold[q_rows] * SCALE
    bq = b_qkv[q_rows] * SCALE
    wk = w_fold[k_rows]
    bk = b_qkv[k_rows]
    wvm = w_fold[v_rows]
    bvm = b_qkv[v_rows]

    # wqk: [c, o] with o = [q(512), k(512)] -> [128, cc, 1024]
    wqkT = np.concatenate([wq, wk], axis=0).T.astype(bf16)          # [512, 1024]
    wqk_t = np.ascontiguousarray(wqkT.reshape(4, 128, 1024).transpose(1, 0, 2))
    bqk_t = np.concatenate([bq, bk]).reshape(8, 128).T.astype(f32)  # [128, 8]
    bqk_t = np.ascontiguousarray(bqk_t)

    wv_t = np.ascontiguousarray(
        wvm.T.astype(bf16).reshape(4, 128, H * VD).transpose(1, 0, 2)
    )
    bv_t = np.ascontiguousarray(
        np.broadcast_to(bvm.astype(bf16)[None, :], (128, H * VD))
    )

    s_p = (inputs["proj_gamma"] / np.sqrt(inputs["proj_var"] + EPS)).astype(f32)
    b_p = (inputs["proj_beta"] - inputs["proj_mean"] * s_p).astype(f32)
    wp_fold = (inputs["proj_w"] * s_p[:, None]).astype(f32)          # [512, 2048]
    wp_t = np.ascontiguousarray(
        wp_fold.T.astype(bf16).reshape(16, 128, DIM).transpose(1, 0, 2)
    )
    bp_t = np.ascontiguousarray(
        np.broadcast_to(b_p.astype(bf16)[None, :], (128, DIM))
    )

    bias_full = inputs["attention_biases"][:, inputs["bias_idxs"]].astype(f32)  # [H, N, N]
    biast = np.zeros((H, NJP, N), dtype=bf16)
    biast[:, :N, :] = np.exp(bias_full).astype(bf16)   # multiplicative form

    xT = inputs["x"].transpose(0, 2, 1).astype(bf16)                 # [B, 512, 784]

    shared = {
        "wqk": wqk_t, "wv": wv_t, "wp": wp_t, "bqk": bqk_t,
        "bv": bv_t, "bp": bp_t, "biast": biast,
        "ident": np.eye(128, dtype=np.float32).astype(bf16),
    }
    in_maps = []
    for c in range(NCORES):
        m = dict(shared)
        m["xT"] = np.ascontiguousarray(xT[c * BL:(c + 1) * BL])
        in_maps.append(m)
    return in_maps


def kernel(trace=False, **inputs):
    from concourse import bass_utils

    if "nc" not in _CACHE:
        _CACHE["nc"] = _build_nc()
    nc = _CACHE["nc"]

    in_maps = _prep(inputs)
    res = bass_utils.run_bass_kernel_spmd(
        nc, in_maps, core_ids=list(range(NCORES)), trace=trace,
    )
    out = np.concatenate([r["out"] for r in res.results], axis=0)
    if trace:
        return out.astype(np.float32), res
    return out.astype(np.float32)


# revision 56
# speedup vs baseline: 1.1976x; 1.0049x over previous
# Fused attention block (LeViT-style) for Trainium2, 8 NeuronCores, data-parallel over batch.
#
# reference computation (B=16, N=784, DIM=512, H=8, KD=64, VD=256):
#   qkv = BN(x @ qkv_w.T); split q,k,v per head
#   attn = softmax(q @ k.T * KD**-0.5 + attention_biases[:, bias_idxs])
#   out  = BN(silu(attn @ v reshaped) @ proj_w.T)
#
# Strategy:
#  - batch-parallel: 2 batches per core, weights/bias tables replicated, no collectives
#  - BN folded into weights on host; softmax scale folded into q weights
#  - all matmul operands bf16 (PSUM accumulation fp32), softmax pipeline fp32
#  - scores computed transposed (S^T[j,i]); bias table is symmetric so bias adds unchanged
#  - softmax denominator from an extra ones-column in v (col 256 of each head block)
#  - unstabilized softmax (scores empirically bounded ~|10|, exp is safe in fp32)
#  - every attention matmul runs at uniform PE tile config (128,128): q/k heads are
#    embedded in zero-padded 128-partition blocks (even head rows 0:64, odd rows
#    64:128), and the j/i tails are zero-padded to 896 so no (64,*) or (*,32)
#    configs appear (config switches cost ~100-300ns each)
#  - exp(S)*exp(bias) == exp(S+bias): host precomputes exp(bias) table; the
#    multiply is split across DVE and the otherwise-idle Pool/GpSimd engine
#  - pass A of batch 1 is emitted between the head-pair phases of batch 0's
#    attention, and batch 0's projection chunks alternate with batch 1's pass B,
#    so the PE has filler work during exp-gated pipeline gaps
#  - attention-phase softmax work is spread across engines: exp on ACT, bias
#    multiply split 9:5 DVE/Pool (non-in-place; in-place DVE RMW is ~2x slower),
#    normalize + psum evictions on DVE, proj transpose evicts alternate ACT/DVE
#  - silu applied per t-chunk at the head of each proj chunk (table loads stay 4)

import numpy as np
import ml_dtypes

B, N, DIM = 16, 784, 512
H, KD, VD = 8, 64, 256
RES = 28
EPS = 1e-5
SCALE = KD ** -0.5
NCORES = 8
BL = B // NCORES          # batches per core
VDA = VD + 1              # v head block with ones column
OVW = H * VDA             # 2056
NJP = 896                 # padded j extent (7 * 128)

# t/j chunking over N=784: six 128-chunks + one 16-chunk
CHUNKS = [(i * 128, min(128, N - i * 128)) for i in range((N + 127) // 128)]
ITILES = [(0, 512), (512, N - 512)]   # free-dim tiles for 784 (<=512 per PSUM bank)

_CACHE = {}


def _build_nc():
    from contextlib import ExitStack
    import concourse.bacc as bacc
    import concourse.tile as tile
    from concourse import mybir

    bf = mybir.dt.bfloat16
    f8 = mybir.dt.float8e4
    f32 = mybir.dt.float32
    AF = mybir.ActivationFunctionType
    MULT = mybir.AluOpType.mult
    ADD = mybir.AluOpType.add

    nc = bacc.Bacc("TRN2", target_bir_lowering=False, debug=False)

    xT = nc.dram_tensor("xT", [BL, DIM, N], bf, kind="ExternalInput").ap()
    wqk = nc.dram_tensor("wqk", [128, 4, 1024], bf, kind="ExternalInput").ap()
    wv = nc.dram_tensor("wv", [128, 4, H * VD], bf, kind="ExternalInput").ap()
    wp = nc.dram_tensor("wp", [128, 16, DIM], bf, kind="ExternalInput").ap()
    bqk = nc.dram_tensor("bqk", [128, 8], f32, kind="ExternalInput").ap()
    bv = nc.dram_tensor("bv", [128, H * VD], bf, kind="ExternalInput").ap()
    bp = nc.dram_tensor("bp", [128, DIM], bf, kind="ExternalInput").ap()
    biast = nc.dram_tensor("biast", [H, NJP, N], bf, kind="ExternalInput").ap()
    ident = nc.dram_tensor("ident", [128, 128], bf, kind="ExternalInput").ap()
    out = nc.dram_tensor("out", [BL, N, DIM], f32, kind="ExternalOutput").ap()

    import concourse.bass as bass

    with ExitStack() as ctx:
        tc = ctx.enter_context(tile.TileContext(nc))
        consts = ctx.enter_context(tc.tile_pool(name="consts", bufs=1))
        xpool = ctx.enter_context(tc.tile_pool(name="xpool", bufs=1))
        qkpool = ctx.enter_context(tc.tile_pool(name="qkpool", bufs=1))
        tevpool = ctx.enter_context(tc.tile_pool(name="tevpool", bufs=3))
        vpool = ctx.enter_context(tc.tile_pool(name="vpool", bufs=1))
        silupool = ctx.enter_context(tc.tile_pool(name="silupool", bufs=1))
        biaspool = ctx.enter_context(tc.tile_pool(name="biaspool", bufs=6))
        eppool = ctx.enter_context(tc.tile_pool(name="eppool", bufs=4))
        ppool = ctx.enter_context(tc.tile_pool(name="ppool", bufs=3))
        smalls = ctx.enter_context(tc.tile_pool(name="smalls", bufs=4))
        tpool = ctx.enter_context(tc.tile_pool(name="tpool", bufs=4))
        fopool = ctx.enter_context(tc.tile_pool(name="fopool", bufs=2))
        # PSUM budget (8 banks): psbig 3x2 + pssm 2x1 = 8
        psbig = ctx.enter_context(tc.tile_pool(name="psbig", bufs=3, space="PSUM"))
        pssm = ctx.enter_context(tc.tile_pool(name="pssm", bufs=2, space="PSUM"))

        # ---- x(0) and wqk first, split per contraction chunk so the first
        # pass A matmuls start as soon as their slices land ----
        x_sb = [None, None]
        x_sb[0] = xpool.tile([128, 4, N], bf, name="x_0", tag="x")
        wqk_sb = consts.tile([128, 4, 1024], bf)
        for cc in range(4):
            nc.sync.dma_start(
                out=x_sb[0][:, cc, :],
                in_=bass.AP(tensor=xT.tensor, offset=xT.offset + cc * 128 * N,
                            ap=[[N, 128], [1, N]]),
            )
            nc.sync.dma_start(out=wqk_sb[:, cc, :], in_=wqk[:, cc, :])
        wv_sb = consts.tile([128, 4, H * VD], bf)
        nc.sync.dma_start(out=wv_sb, in_=wv)
        wp_sb = consts.tile([128, 16, DIM], bf)
        nc.sync.dma_start(out=wp_sb, in_=wp)
        bqk_sb = consts.tile([128, 8], f32)
        nc.sync.dma_start(out=bqk_sb, in_=bqk)
        bp_sb = consts.tile([128, DIM], bf)
        nc.sync.dma_start(out=bp_sb, in_=bp)
        ident_sb = consts.tile([128, 128], bf)
        nc.sync.dma_start(out=ident_sb, in_=ident)
        bv_sb = consts.tile([128, H * VD], bf)
        nc.sync.dma_start(out=bv_sb, in_=bv)

        # qk_pad[*, 2h, :] = q_h, qk_pad[*, 2h+1, :] = k_h; head h occupies
        # partitions (h%2)*64:(h%2+1)*64, everything else stays zero so all
        # score matmuls are uniform K=128, with j-windows up to 896 (cols
        # 784:896 zero) so the j-tail is M=128 too.
        qk_pad = qkpool.tile([128, 16, NJP], bf)
        nc.gpsimd.memset(qk_pad, 0.0)

        # v_sb allocated once: tail rows 16:128 of the last t-chunk are never
        # written by pass B; zero them once so K=128 AV j-tail matmuls stream
        # 0*0 instead of 0*garbage (avoids NaN propagation).
        v_sb = vpool.tile([128, 7, OVW], bf)
        v_resh = v_sb.rearrange("p t (h d) -> p t h d", d=VDA)
        nc.gpsimd.memset(v_sb[:, 6, :], 0.0)
        nc.gpsimd.memset(v_resh[:, :, :, VD:VDA], 1.0)

        silu_sb = [None, None]

        def load_x(b):
            x_t = xpool.tile([128, 4, N], bf, name=f"x_{b}", tag="x")
            xin = bass.AP(
                tensor=xT.tensor,
                offset=xT.offset + b * DIM * N,
                ap=[[N, 128], [128 * N, 4], [1, N]],
            )
            nc.sync.dma_start(out=x_t, in_=xin)
            x_sb[b] = x_t

        def pass_a_oc(b, oc, pool):
            # one o-chunk of qkT: evict to transient, DMA the two head-halves
            # into their zero-padded qk_pad blocks (same partition range)
            tev = tevpool.tile([128, N], bf, name=f"tev_{b}_{oc}", tag="tev")
            ps = pool.tile([128, N], f32, name=f"pa_{b}_{oc}", tag="big")
            for (i0, isz) in ITILES:
                for cc in range(4):
                    nc.tensor.matmul(
                        ps[:, i0:i0 + isz],
                        lhsT=wqk_sb[:, cc, oc * 128:(oc + 1) * 128],
                        rhs=x_sb[b][:, cc, i0:i0 + isz],
                        start=(cc == 0),
                        stop=(cc == 3),
                    )
            nc.vector.tensor_scalar_add(
                out=tev, in0=ps, scalar1=bqk_sb[:, oc:oc + 1],
            )
            isq = 0 if oc < 4 else 1  # q chunks first, then k
            for half in range(2):
                h = (oc % 4) * 2 + half
                blk = 2 * h + isq
                nc.sync.dma_start(
                    out=qk_pad[half * 64:(half + 1) * 64, blk, :N],
                    in_=tev[half * 64:(half + 1) * 64, :],
                )

        def pass_b_ovt(b, ovt):
                # all 7 t-chunks for one head-pair column group; writes only
                # the v regions that AV(b-1, hp=ovt) has finished reading
                o0 = ovt * 512
                for tc_i, (t0, tsz) in enumerate(CHUNKS):
                    ps = pssm.tile([128, 512], f32, tag="small")
                    for cc in range(4):
                        nc.tensor.matmul(
                            ps[:tsz, :],
                            lhsT=x_sb[b][:, cc, t0:t0 + tsz],
                            rhs=wv_sb[:, cc, o0:o0 + 512],
                            start=(cc == 0),
                            stop=(cc == 3),
                        )
                    nc.vector.tensor_tensor(
                        out=v_resh[:tsz, tc_i, 2 * ovt:2 * ovt + 2, :VD],
                        in0=ps[:tsz, :].rearrange("p (h d) -> p h d", d=VD),
                        in1=bv_sb[:tsz, o0:o0 + 512].rearrange(
                            "p (h d) -> p h d", d=VD
                        ),
                        op=ADD,
                    )

        def pass_b_tc(b, tc_i):
                t0, tsz = CHUNKS[tc_i]
                for ovt in range(4):
                    o0 = ovt * 512
                    ps = pssm.tile([128, 512], f32, tag="small")
                    for cc in range(4):
                        nc.tensor.matmul(
                            ps[:tsz, :],
                            lhsT=x_sb[b][:, cc, t0:t0 + tsz],
                            rhs=wv_sb[:, cc, o0:o0 + 512],
                            start=(cc == 0),
                            stop=(cc == 3),
                        )
                    # strided evict into the two 257-stride head blocks; bias
                    # added here (DVE) instead of a K=1 seed matmul on the PE
                    nc.vector.tensor_tensor(
                        out=v_resh[:tsz, tc_i, 2 * ovt:2 * ovt + 2, :VD],
                        in0=ps[:tsz, :].rearrange("p (h d) -> p h d", d=VD),
                        in1=bv_sb[:tsz, o0:o0 + 512].rearrange(
                            "p (h d) -> p h d", d=VD
                        ),
                        op=ADD,
                    )

        def attn_hp(b, hp, p_tail_memset):
            p_par = []
            for k in range(2):
                p_sb = ppool.tile([128, 7, NJP], bf, name=f"p_{b}_{hp}_{k}", tag="p")
                p_par.append(p_sb)
                # zero the i-tail cols once per physical buffer (3-buf round
                # robin; first three allocations cover all bufs) so AV's last
                # i-chunk can run M=128 over the padded window
                if p_tail_memset and (hp, k) in ((0, 0), (0, 1), (1, 0)):
                    nc.gpsimd.memset(p_sb[:, :, N:NJP], 0.0)

            for jc, (j0, jsz) in enumerate(CHUNKS):
                pse = psbig.tile([128, N], f32, tag="big")
                pso = psbig.tile([128, N], f32, tag="big")
                for (i0, isz) in ITILES:
                    for k, ps in ((0, pse), (1, pso)):
                        h = 2 * hp + k
                        nc.tensor.matmul(
                            ps[:, i0:i0 + isz],
                            lhsT=qk_pad[:, 2 * h + 1, j0:j0 + 128],
                            rhs=qk_pad[:, 2 * h, i0:i0 + isz],
                            start=True, stop=True,
                        )
                for k, ps in ((0, pse), (1, pso)):
                    h = 2 * hp + k
                    bias_sb = biaspool.tile([128, N], bf, tag="bt")
                    bin_ = bass.AP(
                        tensor=biast.tensor,
                        offset=biast.offset + (h * NJP + jc * 128) * N,
                        ap=[[N, 128], [1, N]],
                    )
                    nc.sync.dma_start(out=bias_sb, in_=bin_)
                    ep = eppool.tile([128, N], bf, tag="ep")
                    # exp(S)*exp(bias) == exp(S+bias); biast holds exp(bias),
                    # rows >= 784 zero so padded j-tail rows of P come out zero
                    nc.scalar.activation(out=ep, in_=ps[:, :], func=AF.Exp)
                    # non-in-place multiply (in-place RMW runs ~2x slower on
                    # DVE); split across DVE and Pool 9:5 per head pair
                    eng = nc.vector if (k == 0 or jc >= 5) else nc.gpsimd
                    eng.tensor_tensor(
                        out=p_par[k][:, jc, :N], in0=ep, in1=bias_sb, op=MULT,
                    )

            for k in range(2):
                h = 2 * hp + k
                for ic, (i0, isz) in enumerate(CHUNKS):
                    ps = pssm.tile([128, 512], f32, tag="small")
                    for jc, (j0, jsz) in enumerate(CHUNKS):
                        # M=128 always (i-tail reads the zero-padded window;
                        # garbage rows of psum are never read)
                        nc.tensor.matmul(
                            ps[:, :VDA],
                            lhsT=p_par[k][:, jc, i0:i0 + 128],
                            rhs=v_sb[:, jc, h * VDA:(h + 1) * VDA],
                            start=(jc == 0),
                            stop=(jc == 6),
                        )
                    rs = smalls.tile([128, 1], f32)
                    nc.vector.reciprocal(out=rs[:isz], in_=ps[:isz, VD:VDA])
                    # normalized pre-silu values (silu applied in bulk later)
                    nc.vector.tensor_scalar_mul(
                        out=silu_sb[b][:isz, ic, h * VD:(h + 1) * VD],
                        in0=ps[:isz, :VD], scalar1=rs[:isz, 0:1],
                    )

        def proj_tc(b, tc_i, unused=None):
            t0, tsz = CHUNKS[tc_i]
            nc.scalar.activation(
                out=silu_sb[b][:tsz, tc_i, :], in_=silu_sb[b][:tsz, tc_i, :],
                func=AF.Silu,
            )
            psf = pssm.tile([128, 512], f32, tag="small")
            for vp in range(8):
                pst = psbig.tile(
                    [128, 2, 128], bf, name=f"pst_{b}_{tc_i}_{vp}", tag="big",
                )
                for k in range(2):
                    vc = vp * 2 + k
                    nc.tensor.transpose(
                        pst[:, k, :tsz],
                        silu_sb[b][:tsz, tc_i, vc * 128:(vc + 1) * 128],
                        ident_sb[:tsz, :tsz],
                    )
                st = tpool.tile([128, 2, 128], bf)
                # psum -> SBUF evict, alternating ACT/DVE
                if vp % 2 == 0:
                    nc.scalar.copy(out=st[:, :, :tsz], in_=pst[:, :, :tsz])
                else:
                    nc.vector.tensor_copy(out=st[:, :, :tsz], in_=pst[:, :, :tsz])
                for k in range(2):
                    vc = vp * 2 + k
                    nc.tensor.matmul(
                        psf[:tsz, :],
                        lhsT=st[:, k, :tsz],
                        rhs=wp_sb[:, vc, :],
                        start=(vc == 0),
                        stop=(vc == 15),
                    )
            fo = fopool.tile([128, DIM], f32)
            nc.vector.tensor_tensor(
                out=fo[:tsz], in0=psf[:tsz], in1=bp_sb[:tsz], op=ADD,
            )
            nc.sync.dma_start(out=out[b, t0:t0 + tsz, :], in_=fo[:tsz])

        # ---- schedule: passA(b1) interleaved into attn(b0) head-pair
        # phases (fills exp-gated PE gaps); rest sequential ----
        for oc in range(8):
            pass_a_oc(0, oc, psbig)
        for tc_i in range(7):
            pass_b_tc(0, tc_i)
        silu_sb[0] = silupool.tile([128, 7, H * VD], bf, name="silu_0", tag="silu")
        load_x(1)
        for hp in range(4):
            attn_hp(0, hp, p_tail_memset=True)
            pass_a_oc(1, hp, psbig)
            pass_a_oc(1, 4 + hp, psbig)
        for tc_i in range(7):
            proj_tc(0, tc_i)
            pass_b_tc(1, tc_i)
        silu_sb[1] = silupool.tile([128, 7, H * VD], bf, name="silu_1", tag="silu")
        for hp in range(4):
            attn_hp(1, hp, p_tail_memset=False)
        for tc_i in range(7):
            proj_tc(1, tc_i)

    nc.finalize()
    return nc


def _prep(inputs):
    bf16 = ml_dtypes.bfloat16
    f8 = ml_dtypes.float8_e4m3fn
    f32 = np.float32
    inputs = {k: np.asarray(v) for k, v in inputs.items()}

    s_qkv = (inputs["qkv_gamma"] / np.sqrt(inputs["qkv_var"] + EPS)).astype(f32)
    b_qkv = (inputs["qkv_beta"] - inputs["qkv_mean"] * s_qkv).astype(f32)
    w_fold = (inputs["qkv_w"] * s_qkv[:, None]).astype(f32)

    rows = np.arange((2 * KD + VD) * H).reshape(H, 2 * KD + VD)
    q_rows = rows[:, :KD].ravel()
    k_rows = rows[:, KD:2 * KD].ravel()
    v_rows = rows[:, 2 * KD:].ravel()

    wq = w_fold[q_rows] * SCALE
    bq = b_qkv[q_rows] * SCALE
    wk = w_fold[k_rows]
    bk = b_qkv[k_rows]
    wvm = w_fold[v_rows]
    bvm = b_qkv[v_rows]

    # wqk: [c, o] with o = [q(512), k(512)] -> [128, cc, 1024]
    wqkT = np.concatenate([wq, wk], axis=0).T.astype(bf16)          # [512, 1024]
    wqk_t = np.ascontiguousarray(wqkT.reshape(4, 128, 1024).transpose(1, 0, 2))
    bqk_t = np.concatenate([bq, bk]).reshape(8, 128).T.astype(f32)  # [128, 8]
    bqk_t = np.ascontiguousarray(bqk_t)

    wv_t = np.ascontiguousarray(
        wvm.T.astype(bf16).reshape(4, 128, H * VD).transpose(1, 0, 2)
    )
    bv_t = np.ascontiguousarray(
        np.broadcast_to(bvm.astype(bf16)[None, :], (128, H * VD))
    )

    s_p = (inputs["proj_gamma"] / np.sqrt(inputs["proj_var"] + EPS)).astype(f32)
    b_p = (inputs["proj_beta"] - inputs["proj_mean"] * s_p).astype(f32)
    wp_fold = (inputs["proj_w"] * s_p[:, None]).astype(f32)          # [512, 2048]
    wp_t = np.ascontiguousarray(
        wp_fold.T.astype(bf16).reshape(16, 128, DIM).transpose(1, 0, 2)
    )
    bp_t = np.ascontiguousarray(
        np.broadcast_to(b_p.astype(bf16)[None, :], (128, DIM))
    )

    bias_full = inputs["attention_biases"][:, inputs["bias_idxs"]].astype(f32)  # [H, N, N]
    biast = np.zeros((H, NJP, N), dtype=bf16)
    biast[:, :N, :] = np.exp(bias_full).astype(bf16)   # multiplicative form

    xT = inputs["x"].transpose(0, 2, 1).astype(bf16)                 # [B, 512, 784]

    shared = {
        "wqk": wqk_t, "wv": wv_t, "wp": wp_t, "bqk": bqk_t,
        "bv": bv_t, "bp": bp_t, "biast": biast,
        "ident": np.eye(128, dtype=np.float32).astype(bf16),
    }
    in_maps = []
    for c in range(NCORES):
        m = dict(shared)
        m["xT"] = np.ascontiguousarray(xT[c * BL:(c + 1) * BL])
        in_maps.append(m)
    return in_maps


def kernel(trace=False, **inputs):
    from concourse import bass_utils

    if "nc" not in _CACHE:
        _CACHE["nc"] = _build_nc()
    nc = _CACHE["nc"]

    in_maps = _prep(inputs)
    res = bass_utils.run_bass_kernel_spmd(
        nc, in_maps, core_ids=list(range(NCORES)), trace=trace,
    )
    out = np.concatenate([r["out"] for r in res.results], axis=0)
    if trace:
        return out.astype(np.float32), res
    return out.astype(np.float32)
